# revision 15
# baseline (speedup 1.0000x reference)
"""MDRNN 2D-grid recurrence kernel for 8 Trainium2 NeuronCores.

h[i,j] = tanh(x[i,j] @ w + h[i-1,j]*u0 + h[i,j-1]*u1 + bias)

Strategy (v2):
  - Data-parallel over batch: B=16 -> 2 batch elements per core, run as two
    INDEPENDENT anti-diagonal wavefront chains interleaved on the engines
    (decouples the serial dependency chains; engines stay saturated).
  - fp16 storage for x, w, h; fp32 PSUM/z accumulation.
  - GEMM (w stationary, K=64, fp16) runs ahead of the wavefront into PSUM
    chunks aligned to whole diagonals; the per-channel bias is folded into
    the tanh's per-partition bias operand (no ones-row).
  - Per diagonal d of chain b (C cells):
      PE : psum[:, diag] += diag(u0) @ stage_b[up-slice]   (fp16 matmul)
      PE : psum[:, diag] += diag(u1) @ stage_b[left-slice] (fp16 matmul)
      ACT: stage_b[:, d] = tanh(psum[:, diag] + bias)      (fp16 out)
    The two matmuls pipeline back-to-back on the PE; DVE is unused, so the
    serial chain per diagonal is tanh -> mm,mm -> tanh.
  - stage is gap-padded (1 zero col between diagonals) so up/left reads are
    plain shifted slices with boundary zeros from the gaps.
  - Output DMA per 2048-col segment; host inverse-permutes and casts fp32.
"""

import numpy as np

D1, D2, B, SIN, SOUT = 128, 128, 16, 64, 128
NCORES = 8
BLOC = B // NCORES  # 2 chains per core
ND = D1 + D2 - 1  # 255
NC1 = D1 * D2  # 16384 packed cols per chain
SEG = 2048
CHUNK = 512  # psum bank cols


def _geom():
    geo, pb, gb = [], [0], [1]
    for d in range(ND):
        i0 = max(0, d - (D2 - 1))
        i1 = min(D1 - 1, d)
        C = i1 - i0 + 1
        geo.append((i0, C))
        pb.append(pb[-1] + C)
        gb.append(gb[-1] + C + 1)
    return geo, pb, gb


_GEO, _PB, _GB = _geom()
NCG1 = _GB[-1]  # 16640
NSEG = (NCG1 + SEG - 1) // SEG


def _chunks():
    out = []
    d0 = 0
    while d0 < ND:
        pc0 = _PB[d0]
        d1 = d0
        while d1 + 1 < ND and _PB[d1 + 2] - pc0 <= CHUNK:
            d1 += 1
        out.append((d0, d1, pc0, _PB[d1 + 1] - pc0))
        d0 = d1 + 1
    return out


_CHUNKS = _chunks()
_CHUNK_OF = {}
for _ci, (_a, _b, _, _) in enumerate(_CHUNKS):
    for _d in range(_a, _b + 1):
        _CHUNK_OF[_d] = _ci


def _diag_order():
    I, J = [], []
    for d in range(ND):
        for i in range(max(0, d - (D2 - 1)), min(D1 - 1, d) + 1):
            I.append(i)
            J.append(d - i)
    return np.array(I), np.array(J)


_CACHE = {}


def _build_program():
    if "nc" in _CACHE:
        return _CACHE["nc"]
    import concourse.mybir as mybir
    from concourse import bacc
    import concourse.bass as bass
    from concourse.tile import TileContext

    f32 = mybir.dt.float32
    f16 = mybir.dt.float16


    Tanh = mybir.ActivationFunctionType.Tanh

    nc = bacc.Bacc(None, target_bir_lowering=False)
    xa = [
        nc.dram_tensor(f"xa{b}", (SIN, NC1), f16, kind="ExternalInput")
        for b in range(BLOC)
    ]
    wg = nc.dram_tensor("wg", (SIN, SOUT), f16, kind="ExternalInput")
    wd = nc.dram_tensor("wd", (SOUT, SOUT), f16, kind="ExternalInput")
    wd1 = nc.dram_tensor("wd1", (SOUT, SOUT), f16, kind="ExternalInput")
    uvb = nc.dram_tensor("uvb", (SOUT, 2), f32, kind="ExternalInput")
    ho = [
        nc.dram_tensor(f"ho{b}", (SOUT, NCG1), f16, kind="ExternalOutput")
        for b in range(BLOC)
    ]

    XSEG = 2048  # x input DMA segment

    with TileContext(nc) as tc:
        with (
            tc.tile_pool(name="const", bufs=1) as constp,
            tc.tile_pool(name="work", bufs=1) as workp,
            tc.tile_pool(name="psum", bufs=8, space=bass.MemorySpace.PSUM) as psump,
        ):
            wg_sb = constp.tile([SIN, SOUT], f16, tag="wg")
            nc.sync.dma_start(wg_sb[:], wg[:])
            wd_sb = constp.tile([SOUT, SOUT], f16, tag="wd")
            nc.sync.dma_start(wd_sb[:], wd[:])
            wd1_sb = constp.tile([SOUT, SOUT], f16, tag="wd1")
            nc.sync.dma_start(wd1_sb[:], wd1[:])
            u_sb = constp.tile([SOUT, 2], f32, tag="uvb")
            nc.sync.dma_start(u_sb[:], uvb[:])
            bias = u_sb[:, 1:2]

            x_sb, stage = [], []
            for b in range(BLOC):
                xt = constp.tile([SIN, NC1], f16, tag=f"x{b}", name=f"x_sb{b}")
                for s in range(0, NC1, XSEG):
                    nc.sync.dma_start(xt[:, s : s + XSEG], xa[b][:, s : s + XSEG])
                x_sb.append(xt)
                st = workp.tile([SOUT, NCG1], f16, tag=f"st{b}", name=f"stage{b}")
                for s in range(NSEG):
                    lo = s * SEG
                    hi = min(lo + SEG, NCG1)
                    nc.gpsimd.memset(st[:, lo:hi], 0.0)
                stage.append(st)

            # gemm chunk emission (ahead of the wavefront)
            pstile = [[None] * len(_CHUNKS) for _ in range(BLOC)]

            def emit_chunk(b, ci):
                if pstile[b][ci] is not None:
                    return
                _, _, pc0, ncols = _CHUNKS[ci]
                ps = psump.tile([SOUT, CHUNK], f32, tag="ps", name="ps")
                nc.tensor.matmul(
                    out=ps[:, :ncols],
                    lhsT=wg_sb[:],
                    rhs=x_sb[b][:, pc0 : pc0 + ncols],
                    start=True,
                    stop=False,
                )
                pstile[b][ci] = ps

            for b in range(BLOC):
                emit_chunk(b, 0)
                emit_chunk(b, 1)

            seg_done = [0] * BLOC
            for d in range(ND):
                i0, C = _GEO[d]
                gbd = _GB[d]
                if d == 0:
                    hls = hus = 0
                elif _GEO[d - 1][0] == i0:
                    hls = _GB[d - 1]
                    hus = _GB[d - 1] - 1
                else:
                    hls = _GB[d - 1] + 1
                    hus = _GB[d - 1]
                ci = _CHUNK_OF[d]
                poff = _PB[d] - _CHUNKS[ci][2]
                for b in range(BLOC):
                    ps = pstile[b][ci]
                    nc.tensor.matmul(
                        out=ps[:, poff : poff + C],
                        lhsT=wd_sb[:],
                        rhs=stage[b][:, hus : hus + C],
                        start=False,
                        stop=False,
                        skip_group_check=True,
                    )
                    nc.tensor.matmul(
                        out=ps[:, poff : poff + C],
                        lhsT=wd1_sb[:],
                        rhs=stage[b][:, hls : hls + C],
                        start=False,
                        stop=True,
                        skip_group_check=True,
                    )
                    nc.scalar.activation(
                        out=stage[b][:, gbd : gbd + C],
                        in_=ps[:, poff : poff + C],
                        func=Tanh,
                        bias=bias,
                    )
                # prefetch next gemm chunk + flush finished output segments
                if ci + 1 < len(_CHUNKS) and _CHUNK_OF[d] != _CHUNK_OF.get(d + 1, -1):
                    for b in range(BLOC):
                        emit_chunk(b, ci + 1)
                for b in range(BLOC):
                    while (seg_done[b] + 1) * SEG <= gbd:
                        lo = seg_done[b] * SEG
                        nc.sync.dma_start(
                            ho[b][:, lo : lo + SEG], stage[b][:, lo : lo + SEG]
                        )
                        seg_done[b] += 1
            for b in range(BLOC):
                while seg_done[b] * SEG < NCG1:
                    lo = seg_done[b] * SEG
                    hi = min(lo + SEG, NCG1)
                    nc.sync.dma_start(ho[b][:, lo:hi], stage[b][:, lo:hi])
                    seg_done[b] += 1

    nc.compile()
    _CACHE["nc"] = nc
    return nc


def _prep_inputs(x, w, u, bias):
    I, J = _diag_order()
    xd = np.ascontiguousarray(x[I, J])  # (16384, B, SIN) fp32
    wg = w.astype(np.float16)
    wd = np.diag(u[0]).astype(np.float16)
    wd1 = np.diag(u[1]).astype(np.float16)
    uvb = np.stack([u[1], bias], axis=1).astype(np.float32)  # (128, 2)
    in_maps = []
    for c in range(NCORES):
        m = {"wg": wg, "wd": wd, "wd1": wd1, "uvb": uvb}
        for b in range(BLOC):
            xc = xd[:, BLOC * c + b, :]  # (16384, 64)
            m[f"xa{b}"] = np.ascontiguousarray(xc.T.astype(np.float16))
        in_maps.append(m)
    return in_maps


def _assemble(results):
    I, J = _diag_order()
    valid = np.zeros(NC1, np.int64)
    for d in range(ND):
        C = _GEO[d][1]
        valid[_PB[d] : _PB[d] + C] = _GB[d] + np.arange(C)
    out = np.zeros((D1, D2, B, SOUT), np.float32)
    for c in range(NCORES):
        for b in range(BLOC):
            hoc = results[c][f"ho{b}"][:, valid]  # (128, 16384) fp16
            out[I, J, BLOC * c + b, :] = hoc.T.astype(np.float32)
    return out


def kernel(x, w, u, bias, _trace=False):
    from concourse.bass_utils import run_bass_kernel_spmd

    x = np.asarray(x, dtype=np.float32)
    w = np.asarray(w, dtype=np.float32)
    u = np.asarray(u, dtype=np.float32)
    bias = np.asarray(bias, dtype=np.float32)

    nc = _build_program()
    in_maps = _prep_inputs(x, w, u, bias)
    res = run_bass_kernel_spmd(
        nc, in_maps, core_ids=list(range(NCORES)), trace=_trace
    )
    _CACHE["last_result"] = res
    return _assemble(res.results)


# revision 17
# speedup vs baseline: 1.0718x; 1.0718x over previous
"""MDRNN 2D-grid recurrence kernel for 8 Trainium2 NeuronCores.

h[i,j] = tanh(x[i,j] @ w + h[i-1,j]*u0 + h[i,j-1]*u1 + bias)

Strategy (v2):
  - Data-parallel over batch: B=16 -> 2 batch elements per core, run as two
    INDEPENDENT anti-diagonal wavefront chains interleaved on the engines
    (decouples the serial dependency chains; engines stay saturated).
  - fp16 storage for x, w, h; fp32 PSUM/z accumulation.
  - GEMM (w stationary, K=64, fp16) runs ahead of the wavefront into PSUM
    chunks aligned to whole diagonals; the per-channel bias is folded into
    the tanh's per-partition bias operand (no ones-row).
  - Per diagonal d of chain b (C cells):
      PE : psum[:, diag] += diag(u0) @ stage_b[up-slice]   (fp16 matmul)
      PE : psum[:, diag] += diag(u1) @ stage_b[left-slice] (fp16 matmul)
      ACT: stage_b[:, d] = tanh(psum[:, diag] + bias)      (fp16 out)
    The two matmuls pipeline back-to-back on the PE; DVE is unused, so the
    serial chain per diagonal is tanh -> mm,mm -> tanh.
  - stage is gap-padded (1 zero col between diagonals) so up/left reads are
    plain shifted slices with boundary zeros from the gaps.
  - Output DMA per 2048-col segment; host inverse-permutes and casts fp32.
"""

import numpy as np

D1, D2, B, SIN, SOUT = 128, 128, 16, 64, 128
NCORES = 8
BLOC = B // NCORES  # 2 chains per core
ND = D1 + D2 - 1  # 255
NC1 = D1 * D2  # 16384 packed cols per chain
SEG = 2048
CHUNK = 512  # psum bank cols


def _geom():
    geo, pb, gb = [], [0], [1]
    for d in range(ND):
        i0 = max(0, d - (D2 - 1))
        i1 = min(D1 - 1, d)
        C = i1 - i0 + 1
        geo.append((i0, C))
        pb.append(pb[-1] + C)
        gb.append(gb[-1] + C + 1)
    return geo, pb, gb


_GEO, _PB, _GB = _geom()
NCG1 = _GB[-1]  # 16640
NSEG = (NCG1 + SEG - 1) // SEG


def _chunks():
    out = []
    d0 = 0
    while d0 < ND:
        pc0 = _PB[d0]
        d1 = d0
        while d1 + 1 < ND and _PB[d1 + 2] - pc0 <= CHUNK:
            d1 += 1
        out.append((d0, d1, pc0, _PB[d1 + 1] - pc0))
        d0 = d1 + 1
    return out


_CHUNKS = _chunks()
_CHUNK_OF = {}
for _ci, (_a, _b, _, _) in enumerate(_CHUNKS):
    for _d in range(_a, _b + 1):
        _CHUNK_OF[_d] = _ci


def _diag_order():
    I, J = [], []
    for d in range(ND):
        for i in range(max(0, d - (D2 - 1)), min(D1 - 1, d) + 1):
            I.append(i)
            J.append(d - i)
    return np.array(I), np.array(J)


_CACHE = {}


def _build_program():
    if "nc" in _CACHE:
        return _CACHE["nc"]
    import concourse.mybir as mybir
    from concourse import bacc
    import concourse.bass as bass
    from concourse.tile import TileContext

    f32 = mybir.dt.float32
    f16 = mybir.dt.float16


    Tanh = mybir.ActivationFunctionType.Tanh

    nc = bacc.Bacc(None, target_bir_lowering=False)
    xa = [
        nc.dram_tensor(f"xa{b}", (SIN, NC1), f16, kind="ExternalInput")
        for b in range(BLOC)
    ]
    wg = nc.dram_tensor("wg", (SIN, SOUT), f16, kind="ExternalInput")
    wd = nc.dram_tensor("wd", (SOUT, SOUT), f16, kind="ExternalInput")
    wd1 = nc.dram_tensor("wd1", (SOUT, SOUT), f16, kind="ExternalInput")
    uvb = nc.dram_tensor("uvb", (SOUT, 2), f32, kind="ExternalInput")
    ho = [
        nc.dram_tensor(f"ho{b}", (SOUT, NCG1), f16, kind="ExternalOutput")
        for b in range(BLOC)
    ]

    XSEG = 2048  # x input DMA segment

    with TileContext(nc) as tc:
        with (
            tc.tile_pool(name="const", bufs=1) as constp,
            tc.tile_pool(name="work", bufs=1) as workp,
            tc.tile_pool(name="psum", bufs=8, space=bass.MemorySpace.PSUM) as psump,
        ):
            # DMA order matters for ramp: wg + the first small x pieces go
            # first so gemm chunk 0 can start ASAP; wd/wd1/uvb follow (they
            # gate only the first recurrence matmul / tanh).
            wg_sb = constp.tile([SIN, SOUT], f16, tag="wg")
            nc.sync.dma_start(wg_sb[:], wg[:])
            wd_sb = constp.tile([SOUT, SOUT], f16, tag="wd")
            wd1_sb = constp.tile([SOUT, SOUT], f16, tag="wd1")
            u_sb = constp.tile([SOUT, 2], f32, tag="uvb")
            bias = u_sb[:, 1:2]

            x_sb, stage = [], []
            for b in range(BLOC):
                xt = constp.tile([SIN, NC1], f16, tag=f"x{b}", name=f"x_sb{b}")
                x_sb.append(xt)
                st = workp.tile([SOUT, NCG1], f16, tag=f"st{b}", name=f"stage{b}")
                stage.append(st)
            for b in range(BLOC):
                nc.sync.dma_start(x_sb[b][:, 0:512], xa[b][:, 0:512])
            nc.sync.dma_start(wd_sb[:], wd[:])
            nc.sync.dma_start(wd1_sb[:], wd1[:])
            nc.sync.dma_start(u_sb[:], uvb[:])
            xsegs = [(512, XSEG)] + [
                (s, s + XSEG) for s in range(XSEG, NC1, XSEG)
            ]
            for lo, hi in xsegs:
                for b in range(BLOC):
                    nc.sync.dma_start(x_sb[b][:, lo:hi], xa[b][:, lo:hi])
            for s in range(NSEG):
                lo = s * SEG
                hi = min(lo + SEG, NCG1)
                for b in range(BLOC):
                    nc.gpsimd.memset(stage[b][:, lo:hi], 0.0)

            # gemm chunk emission (ahead of the wavefront)
            pstile = [[None] * len(_CHUNKS) for _ in range(BLOC)]

            def emit_chunk(b, ci):
                if pstile[b][ci] is not None:
                    return
                _, _, pc0, ncols = _CHUNKS[ci]
                ps = psump.tile([SOUT, CHUNK], f32, tag="ps", name="ps")
                nc.tensor.matmul(
                    out=ps[:, :ncols],
                    lhsT=wg_sb[:],
                    rhs=x_sb[b][:, pc0 : pc0 + ncols],
                    start=True,
                    stop=False,
                )
                pstile[b][ci] = ps

            for b in range(BLOC):
                emit_chunk(b, 0)
                emit_chunk(b, 1)

            seg_done = [0] * BLOC
            for d in range(ND):
                i0, C = _GEO[d]
                gbd = _GB[d]
                if d == 0:
                    hls = hus = 0
                elif _GEO[d - 1][0] == i0:
                    hls = _GB[d - 1]
                    hus = _GB[d - 1] - 1
                else:
                    hls = _GB[d - 1] + 1
                    hus = _GB[d - 1]
                ci = _CHUNK_OF[d]
                poff = _PB[d] - _CHUNKS[ci][2]
                crossing = ci + 1 < len(_CHUNKS) and _CHUNK_OF[d] != _CHUNK_OF.get(
                    d + 1, -1
                )
                for b in range(BLOC):
                    ps = pstile[b][ci]
                    nc.tensor.matmul(
                        out=ps[:, poff : poff + C],
                        lhsT=wd_sb[:],
                        rhs=stage[b][:, hus : hus + C],
                        start=False,
                        stop=False,
                        skip_group_check=True,
                    )
                    nc.tensor.matmul(
                        out=ps[:, poff : poff + C],
                        lhsT=wd1_sb[:],
                        rhs=stage[b][:, hls : hls + C],
                        start=False,
                        stop=True,
                        skip_group_check=True,
                    )
                    # prefetch this chain's next gemm chunk in the PE idle
                    # window right behind this chain's recurrence matmuls
                    if crossing:
                        emit_chunk(b, ci + 1)
                    nc.scalar.activation(
                        out=stage[b][:, gbd : gbd + C],
                        in_=ps[:, poff : poff + C],
                        func=Tanh,
                        bias=bias,
                    )
                for b in range(BLOC):
                    while (seg_done[b] + 1) * SEG <= gbd:
                        lo = seg_done[b] * SEG
                        nc.sync.dma_start(
                            ho[b][:, lo : lo + SEG], stage[b][:, lo : lo + SEG]
                        )
                        seg_done[b] += 1
            for b in range(BLOC):
                while seg_done[b] * SEG < NCG1:
                    lo = seg_done[b] * SEG
                    hi = min(lo + SEG, NCG1)
                    nc.sync.dma_start(ho[b][:, lo:hi], stage[b][:, lo:hi])
                    seg_done[b] += 1

    nc.compile()
    _CACHE["nc"] = nc
    return nc


def _prep_inputs(x, w, u, bias):
    I, J = _diag_order()
    xd = np.ascontiguousarray(x[I, J])  # (16384, B, SIN) fp32
    wg = w.astype(np.float16)
    wd = np.diag(u[0]).astype(np.float16)
    wd1 = np.diag(u[1]).astype(np.float16)
    uvb = np.stack([u[1], bias], axis=1).astype(np.float32)  # (128, 2)
    in_maps = []
    for c in range(NCORES):
        m = {"wg": wg, "wd": wd, "wd1": wd1, "uvb": uvb}
        for b in range(BLOC):
            xc = xd[:, BLOC * c + b, :]  # (16384, 64)
            m[f"xa{b}"] = np.ascontiguousarray(xc.T.astype(np.float16))
        in_maps.append(m)
    return in_maps


def _assemble(results):
    I, J = _diag_order()
    valid = np.zeros(NC1, np.int64)
    for d in range(ND):
        C = _GEO[d][1]
        valid[_PB[d] : _PB[d] + C] = _GB[d] + np.arange(C)
    out = np.zeros((D1, D2, B, SOUT), np.float32)
    for c in range(NCORES):
        for b in range(BLOC):
            hoc = results[c][f"ho{b}"][:, valid]  # (128, 16384) fp16
            out[I, J, BLOC * c + b, :] = hoc.T.astype(np.float32)
    return out


def kernel(x, w, u, bias, _trace=False):
    from concourse.bass_utils import run_bass_kernel_spmd

    x = np.asarray(x, dtype=np.float32)
    w = np.asarray(w, dtype=np.float32)
    u = np.asarray(u, dtype=np.float32)
    bias = np.asarray(bias, dtype=np.float32)

    nc = _build_program()
    in_maps = _prep_inputs(x, w, u, bias)
    res = run_bass_kernel_spmd(
        nc, in_maps, core_ids=list(range(NCORES)), trace=_trace
    )
    _CACHE["last_result"] = res
    return _assemble(res.results)


# revision 19
# speedup vs baseline: 1.0891x; 1.0161x over previous
"""MDRNN 2D-grid recurrence kernel for 8 Trainium2 NeuronCores.

h[i,j] = tanh(x[i,j] @ w + h[i-1,j]*u0 + h[i,j-1]*u1 + bias)

Strategy (v2):
  - Data-parallel over batch: B=16 -> 2 batch elements per core, run as two
    INDEPENDENT anti-diagonal wavefront chains interleaved on the engines
    (decouples the serial dependency chains; engines stay saturated).
  - fp16 storage for x, w, h; fp32 PSUM/z accumulation.
  - GEMM (w stationary, K=64, fp16) runs ahead of the wavefront into PSUM
    chunks aligned to whole diagonals; the per-channel bias is folded into
    the tanh's per-partition bias operand (no ones-row).
  - Per diagonal d of chain b (C cells):
      PE : psum[:, diag] += diag(u0) @ stage_b[up-slice]   (fp16 matmul)
      PE : psum[:, diag] += diag(u1) @ stage_b[left-slice] (fp16 matmul)
      ACT: stage_b[:, d] = tanh(psum[:, diag] + bias)      (fp16 out)
    The two matmuls pipeline back-to-back on the PE; DVE is unused, so the
    serial chain per diagonal is tanh -> mm,mm -> tanh.
  - stage is gap-padded (1 zero col between diagonals) so up/left reads are
    plain shifted slices with boundary zeros from the gaps.
  - Output DMA per 2048-col segment; host inverse-permutes and casts fp32.
"""

import numpy as np

D1, D2, B, SIN, SOUT = 128, 128, 16, 64, 128
NCORES = 8
BLOC = B // NCORES  # 2 chains per core
ND = D1 + D2 - 1  # 255
NC1 = D1 * D2  # 16384 packed cols per chain
SEG = 2048
CHUNK = 512  # psum bank cols


def _geom():
    geo, pb, gb = [], [0], [1]
    for d in range(ND):
        i0 = max(0, d - (D2 - 1))
        i1 = min(D1 - 1, d)
        C = i1 - i0 + 1
        geo.append((i0, C))
        pb.append(pb[-1] + C)
        gb.append(gb[-1] + C + 1)
    return geo, pb, gb


_GEO, _PB, _GB = _geom()
NCG1 = _GB[-1]  # 16640
NSEG = (NCG1 + SEG - 1) // SEG


def _chunks():
    # <=2 diagonals per chunk: the gemm matmul (<=256 cols, ~340ns) then
    # fits inside a single tanh-wait window on the PE and never delays the
    # wavefront's chain matmuls. Each chunk still gets a full 2KB bank tile
    # (PSUM start=True marks the whole bank pending-zero, so chunks must
    # never share a bank).
    out = []
    d0 = 0
    while d0 < ND:
        pc0 = _PB[d0]
        d1 = d0
        if d1 + 1 < ND and _PB[d1 + 2] - pc0 <= CHUNK:
            d1 += 1
        out.append((d0, d1, pc0, _PB[d1 + 1] - pc0))
        d0 = d1 + 1
    return out


_CHUNKS = _chunks()
_CHUNK_OF = {}
for _ci, (_a, _b, _, _) in enumerate(_CHUNKS):
    for _d in range(_a, _b + 1):
        _CHUNK_OF[_d] = _ci


def _diag_order():
    I, J = [], []
    for d in range(ND):
        for i in range(max(0, d - (D2 - 1)), min(D1 - 1, d) + 1):
            I.append(i)
            J.append(d - i)
    return np.array(I), np.array(J)


_CACHE = {}


def _build_program():
    if "nc" in _CACHE:
        return _CACHE["nc"]
    import concourse.mybir as mybir
    from concourse import bacc
    import concourse.bass as bass
    from concourse.tile import TileContext

    f32 = mybir.dt.float32
    f16 = mybir.dt.float16
    Tanh = mybir.ActivationFunctionType.Tanh

    nc = bacc.Bacc(None, target_bir_lowering=False)
    xa = [
        nc.dram_tensor(f"xa{b}", (SIN, NC1), f16, kind="ExternalInput")
        for b in range(BLOC)
    ]
    wg = nc.dram_tensor("wg", (SIN, SOUT), f16, kind="ExternalInput")
    wd = nc.dram_tensor("wd", (SOUT, SOUT), f16, kind="ExternalInput")
    wd1 = nc.dram_tensor("wd1", (SOUT, SOUT), f16, kind="ExternalInput")
    uvb = nc.dram_tensor("uvb", (SOUT, 2), f32, kind="ExternalInput")
    ho = [
        nc.dram_tensor(f"ho{b}", (SOUT, NCG1), f16, kind="ExternalOutput")
        for b in range(BLOC)
    ]

    XSEG = 2048  # x input DMA segment

    with TileContext(nc) as tc:
        with (
            tc.tile_pool(name="const", bufs=1) as constp,
            tc.tile_pool(name="work", bufs=1) as workp,
            tc.tile_pool(name="psum", bufs=8, space=bass.MemorySpace.PSUM) as psump,
        ):
            # DMA order matters for ramp: wg + the first small x pieces go
            # first so gemm chunk 0 can start ASAP; wd/wd1/uvb follow (they
            # gate only the first recurrence matmul / tanh).
            wg_sb = constp.tile([SIN, SOUT], f16, tag="wg")
            nc.sync.dma_start(wg_sb[:], wg[:])
            wd_sb = constp.tile([SOUT, SOUT], f16, tag="wd")
            wd1_sb = constp.tile([SOUT, SOUT], f16, tag="wd1")
            u_sb = constp.tile([SOUT, 2], f32, tag="uvb")
            bias = u_sb[:, 1:2]

            x_sb, stage = [], []
            for b in range(BLOC):
                xt = constp.tile([SIN, NC1], f16, tag=f"x{b}", name=f"x_sb{b}")
                x_sb.append(xt)
                st = workp.tile([SOUT, NCG1], f16, tag=f"st{b}", name=f"stage{b}")
                stage.append(st)
            for b in range(BLOC):
                nc.sync.dma_start(x_sb[b][:, 0:512], xa[b][:, 0:512])
            nc.sync.dma_start(wd_sb[:], wd[:])
            nc.sync.dma_start(wd1_sb[:], wd1[:])
            nc.sync.dma_start(u_sb[:], uvb[:])
            xsegs = [(512, XSEG)] + [
                (s, s + XSEG) for s in range(XSEG, NC1, XSEG)
            ]
            for lo, hi in xsegs:
                for b in range(BLOC):
                    nc.sync.dma_start(x_sb[b][:, lo:hi], xa[b][:, lo:hi])
            for s in range(NSEG):
                lo = s * SEG
                hi = min(lo + SEG, NCG1)
                for b in range(BLOC):
                    nc.gpsimd.memset(stage[b][:, lo:hi], 0.0)

            # gemm chunk emission (ahead of the wavefront)
            pstile = [[None] * len(_CHUNKS) for _ in range(BLOC)]

            def emit_chunk(b, ci):
                if pstile[b][ci] is not None:
                    return
                _, _, pc0, ncols = _CHUNKS[ci]
                ps = psump.tile([SOUT, CHUNK], f32, tag="ps", name="ps")
                nc.tensor.matmul(
                    out=ps[:, :ncols],
                    lhsT=wg_sb[:],
                    rhs=x_sb[b][:, pc0 : pc0 + ncols],
                    start=True,
                    stop=False,
                )
                pstile[b][ci] = ps

            for b in range(BLOC):
                emit_chunk(b, 0)
                emit_chunk(b, 1)

            seg_done = [0] * BLOC
            for d in range(ND):
                i0, C = _GEO[d]
                gbd = _GB[d]
                if d == 0:
                    hls = hus = 0
                elif _GEO[d - 1][0] == i0:
                    hls = _GB[d - 1]
                    hus = _GB[d - 1] - 1
                else:
                    hls = _GB[d - 1] + 1
                    hus = _GB[d - 1]
                ci = _CHUNK_OF[d]
                poff = _PB[d] - _CHUNKS[ci][2]
                crossing = ci + 1 < len(_CHUNKS) and _CHUNK_OF[d] != _CHUNK_OF.get(
                    d + 1, -1
                )
                for b in range(BLOC):
                    ps = pstile[b][ci]
                    nc.tensor.matmul(
                        out=ps[:, poff : poff + C],
                        lhsT=wd_sb[:],
                        rhs=stage[b][:, hus : hus + C],
                        start=False,
                        stop=False,
                        skip_group_check=True,
                    )
                    nc.tensor.matmul(
                        out=ps[:, poff : poff + C],
                        lhsT=wd1_sb[:],
                        rhs=stage[b][:, hls : hls + C],
                        start=False,
                        stop=True,
                        skip_group_check=True,
                    )
                    # prefetch this chain's next gemm chunk in the PE idle
                    # window right behind this chain's recurrence matmuls
                    if crossing:
                        emit_chunk(b, ci + 1)
                    nc.scalar.activation(
                        out=stage[b][:, gbd : gbd + C],
                        in_=ps[:, poff : poff + C],
                        func=Tanh,
                        bias=bias,
                    )
                for b in range(BLOC):
                    while (seg_done[b] + 1) * SEG <= gbd:
                        lo = seg_done[b] * SEG
                        nc.sync.dma_start(
                            ho[b][:, lo : lo + SEG], stage[b][:, lo : lo + SEG]
                        )
                        seg_done[b] += 1
            for b in range(BLOC):
                while seg_done[b] * SEG < NCG1:
                    lo = seg_done[b] * SEG
                    hi = min(lo + SEG, NCG1)
                    nc.sync.dma_start(ho[b][:, lo:hi], stage[b][:, lo:hi])
                    seg_done[b] += 1

    nc.compile()
    _CACHE["nc"] = nc
    return nc


def _prep_inputs(x, w, u, bias):
    I, J = _diag_order()
    xd = np.ascontiguousarray(x[I, J])  # (16384, B, SIN) fp32
    wg = w.astype(np.float16)
    wd = np.diag(u[0]).astype(np.float16)
    wd1 = np.diag(u[1]).astype(np.float16)
    uvb = np.stack([u[1], bias], axis=1).astype(np.float32)  # (128, 2)
    in_maps = []
    for c in range(NCORES):
        m = {"wg": wg, "wd": wd, "wd1": wd1, "uvb": uvb}
        for b in range(BLOC):
            xc = xd[:, BLOC * c + b, :]  # (16384, 64)
            m[f"xa{b}"] = np.ascontiguousarray(xc.T.astype(np.float16))
        in_maps.append(m)
    return in_maps


def _assemble(results):
    I, J = _diag_order()
    valid = np.zeros(NC1, np.int64)
    for d in range(ND):
        C = _GEO[d][1]
        valid[_PB[d] : _PB[d] + C] = _GB[d] + np.arange(C)
    out = np.zeros((D1, D2, B, SOUT), np.float32)
    for c in range(NCORES):
        for b in range(BLOC):
            hoc = results[c][f"ho{b}"][:, valid]  # (128, 16384) fp16
            out[I, J, BLOC * c + b, :] = hoc.T.astype(np.float32)
    return out


def kernel(x, w, u, bias, _trace=False):
    from concourse.bass_utils import run_bass_kernel_spmd

    x = np.asarray(x, dtype=np.float32)
    w = np.asarray(w, dtype=np.float32)
    u = np.asarray(u, dtype=np.float32)
    bias = np.asarray(bias, dtype=np.float32)

    nc = _build_program()
    in_maps = _prep_inputs(x, w, u, bias)
    res = run_bass_kernel_spmd(
        nc, in_maps, core_ids=list(range(NCORES)), trace=_trace
    )
    _CACHE["last_result"] = res
    return _assemble(res.results)


# revision 21
# speedup vs baseline: 1.0961x; 1.0064x over previous
"""MDRNN 2D-grid recurrence kernel for 8 Trainium2 NeuronCores.

h[i,j] = tanh(x[i,j] @ w + h[i-1,j]*u0 + h[i,j-1]*u1 + bias)

Strategy (v2):
  - Data-parallel over batch: B=16 -> 2 batch elements per core, run as two
    INDEPENDENT anti-diagonal wavefront chains interleaved on the engines
    (decouples the serial dependency chains; engines stay saturated).
  - fp16 storage for x, w, h; fp32 PSUM/z accumulation.
  - GEMM (w stationary, K=64, fp16) runs ahead of the wavefront into PSUM
    chunks aligned to whole diagonals; the per-channel bias is folded into
    the tanh's per-partition bias operand (no ones-row).
  - Per diagonal d of chain b (C cells):
      PE : psum[:, diag] += diag(u0) @ stage_b[up-slice]   (fp16 matmul)
      PE : psum[:, diag] += diag(u1) @ stage_b[left-slice] (fp16 matmul)
      ACT: stage_b[:, d] = tanh(psum[:, diag] + bias)      (fp16 out)
    The two matmuls pipeline back-to-back on the PE; DVE is unused, so the
    serial chain per diagonal is tanh -> mm,mm -> tanh.
  - stage is gap-padded (1 zero col between diagonals) so up/left reads are
    plain shifted slices with boundary zeros from the gaps.
  - Output DMA per 2048-col segment; host inverse-permutes and casts fp32.
"""

import numpy as np

D1, D2, B, SIN, SOUT = 128, 128, 16, 64, 128
NCORES = 8
BLOC = B // NCORES  # 2 chains per core
ND = D1 + D2 - 1  # 255
NC1 = D1 * D2  # 16384 packed cols per chain
SEG = 2048
CHUNK = 512  # psum bank cols


def _geom():
    geo, pb, gb = [], [0], [1]
    for d in range(ND):
        i0 = max(0, d - (D2 - 1))
        i1 = min(D1 - 1, d)
        C = i1 - i0 + 1
        geo.append((i0, C))
        pb.append(pb[-1] + C)
        gb.append(gb[-1] + C + 1)
    return geo, pb, gb


_GEO, _PB, _GB = _geom()
NCG1 = _GB[-1]  # 16640
NSEG = (NCG1 + SEG - 1) // SEG


def _chunks():
    # <=2 diagonals per chunk: the gemm matmul (<=256 cols, ~340ns) then
    # fits inside a single tanh-wait window on the PE and never delays the
    # wavefront's chain matmuls. Each chunk still gets a full 2KB bank tile
    # (PSUM start=True marks the whole bank pending-zero, so chunks must
    # never share a bank).
    out = []
    d0 = 0
    while d0 < ND:
        pc0 = _PB[d0]
        d1 = d0
        if d1 + 1 < ND and _PB[d1 + 2] - pc0 <= CHUNK:
            d1 += 1
        out.append((d0, d1, pc0, _PB[d1 + 1] - pc0))
        d0 = d1 + 1
    return out


_CHUNKS = _chunks()
_CHUNK_OF = {}
for _ci, (_a, _b, _, _) in enumerate(_CHUNKS):
    for _d in range(_a, _b + 1):
        _CHUNK_OF[_d] = _ci


def _diag_order():
    I, J = [], []
    for d in range(ND):
        for i in range(max(0, d - (D2 - 1)), min(D1 - 1, d) + 1):
            I.append(i)
            J.append(d - i)
    return np.array(I), np.array(J)


_CACHE = {}


def _build_program():
    if "nc" in _CACHE:
        return _CACHE["nc"]
    import concourse.mybir as mybir
    from concourse import bacc
    import concourse.bass as bass
    from concourse.tile import TileContext

    f32 = mybir.dt.float32
    f16 = mybir.dt.float16
    Tanh = mybir.ActivationFunctionType.Tanh

    nc = bacc.Bacc(None, target_bir_lowering=False)
    xa = [
        nc.dram_tensor(f"xa{b}", (SIN, NC1), f16, kind="ExternalInput")
        for b in range(BLOC)
    ]
    wg = nc.dram_tensor("wg", (SIN, SOUT), f16, kind="ExternalInput")
    wd = nc.dram_tensor("wd", (SOUT, SOUT), f16, kind="ExternalInput")
    wd1 = nc.dram_tensor("wd1", (SOUT, SOUT), f16, kind="ExternalInput")
    uvb = nc.dram_tensor("uvb", (SOUT, 2), f32, kind="ExternalInput")
    ho = [
        nc.dram_tensor(f"ho{b}", (SOUT, NCG1), f16, kind="ExternalOutput")
        for b in range(BLOC)
    ]

    XSEG = 2048  # x input DMA segment

    with TileContext(nc) as tc:
        with (
            tc.tile_pool(name="const", bufs=1) as constp,
            tc.tile_pool(name="work", bufs=1) as workp,
            tc.tile_pool(name="psum", bufs=8, space=bass.MemorySpace.PSUM) as psump,
        ):
            # DMA order matters for ramp: wg + the first small x pieces go
            # first so gemm chunk 0 can start ASAP; wd/wd1/uvb follow (they
            # gate only the first recurrence matmul / tanh).
            wg_sb = constp.tile([SIN, SOUT], f16, tag="wg")
            nc.sync.dma_start(wg_sb[:], wg[:])
            wd_sb = constp.tile([SOUT, SOUT], f16, tag="wd")
            wd1_sb = constp.tile([SOUT, SOUT], f16, tag="wd1")
            u_sb = constp.tile([SOUT, 2], f32, tag="uvb")
            bias = u_sb[:, 1:2]

            x_sb, stage = [], []
            for b in range(BLOC):
                xt = constp.tile([SIN, NC1], f16, tag=f"x{b}", name=f"x_sb{b}")
                x_sb.append(xt)
                st = workp.tile([SOUT, NCG1], f16, tag=f"st{b}", name=f"stage{b}")
                stage.append(st)
            for b in range(BLOC):
                nc.sync.dma_start(x_sb[b][:, 0:512], xa[b][:, 0:512])
            nc.sync.dma_start(wd_sb[:], wd[:])
            nc.sync.dma_start(wd1_sb[:], wd1[:])
            nc.sync.dma_start(u_sb[:], uvb[:])
            xsegs = [(512, XSEG)] + [
                (s, s + XSEG) for s in range(XSEG, NC1, XSEG)
            ]
            for lo, hi in xsegs:
                for b in range(BLOC):
                    nc.sync.dma_start(x_sb[b][:, lo:hi], xa[b][:, lo:hi])
            for s in range(NSEG):
                lo = s * SEG
                hi = min(lo + SEG, NCG1)
                for b in range(BLOC):
                    nc.gpsimd.memset(stage[b][:, lo:hi], 0.0)

            # gemm chunk emission (ahead of the wavefront)
            pstile = [[None] * len(_CHUNKS) for _ in range(BLOC)]

            def emit_chunk(b, ci):
                if pstile[b][ci] is not None:
                    return
                _, _, pc0, ncols = _CHUNKS[ci]
                ps = psump.tile([SOUT, CHUNK], f32, tag="ps", name="ps")
                nc.tensor.matmul(
                    out=ps[:, :ncols],
                    lhsT=wg_sb[:],
                    rhs=x_sb[b][:, pc0 : pc0 + ncols],
                    start=True,
                    stop=False,
                )
                pstile[b][ci] = ps

            for b in range(BLOC):
                emit_chunk(b, 0)
                emit_chunk(b, 1)

            seg_done = [0] * BLOC
            for d in range(ND):
                i0, C = _GEO[d]
                gbd = _GB[d]
                if d == 0:
                    hls = hus = 0
                elif _GEO[d - 1][0] == i0:
                    hls = _GB[d - 1]
                    hus = _GB[d - 1] - 1
                else:
                    hls = _GB[d - 1] + 1
                    hus = _GB[d - 1]
                ci = _CHUNK_OF[d]
                poff = _PB[d] - _CHUNKS[ci][2]
                crossing = ci + 1 < len(_CHUNKS) and _CHUNK_OF[d] != _CHUNK_OF.get(
                    d + 1, -1
                )
                for b in range(BLOC):
                    ps = pstile[b][ci]
                    nc.tensor.matmul(
                        out=ps[:, poff : poff + C],
                        lhsT=wd_sb[:],
                        rhs=stage[b][:, hus : hus + C],
                        start=False,
                        stop=False,
                        skip_group_check=True,
                    )
                    nc.tensor.matmul(
                        out=ps[:, poff : poff + C],
                        lhsT=wd1_sb[:],
                        rhs=stage[b][:, hls : hls + C],
                        start=False,
                        stop=True,
                        skip_group_check=True,
                    )
                    # prefetch this chain's next gemm chunk in the PE idle
                    # window right behind this chain's recurrence matmuls
                    if crossing:
                        emit_chunk(b, ci + 1)
                    nc.scalar.activation(
                        out=stage[b][:, gbd : gbd + C],
                        in_=ps[:, poff : poff + C],
                        func=Tanh,
                        bias=bias,
                    )
                for b in range(BLOC):
                    while (seg_done[b] + 1) * SEG <= gbd:
                        lo = seg_done[b] * SEG
                        nc.sync.dma_start(
                            ho[b][:, lo : lo + SEG], stage[b][:, lo : lo + SEG]
                        )
                        seg_done[b] += 1
            for b in range(BLOC):
                while seg_done[b] * SEG < NCG1:
                    lo = seg_done[b] * SEG
                    hi = min(lo + SEG, NCG1)
                    nc.sync.dma_start(ho[b][:, lo:hi], stage[b][:, lo:hi])
                    seg_done[b] += 1

    nc.compile()
    _CACHE["nc"] = nc
    return nc


def _prep_inputs(x, w, u, bias):
    I, J = _diag_order()
    xd = np.ascontiguousarray(x[I, J])  # (16384, B, SIN) fp32
    wg = w.astype(np.float16)
    wd = np.diag(u[0]).astype(np.float16)
    wd1 = np.diag(u[1]).astype(np.float16)
    uvb = np.stack([u[1], bias], axis=1).astype(np.float32)  # (128, 2)
    in_maps = []
    for c in range(NCORES):
        m = {"wg": wg, "wd": wd, "wd1": wd1, "uvb": uvb}
        for b in range(BLOC):
            xc = xd[:, BLOC * c + b, :]  # (16384, 64)
            m[f"xa{b}"] = np.ascontiguousarray(xc.T.astype(np.float16))
        in_maps.append(m)
    return in_maps


def _assemble(results):
    I, J = _diag_order()
    valid = np.zeros(NC1, np.int64)
    for d in range(ND):
        C = _GEO[d][1]
        valid[_PB[d] : _PB[d] + C] = _GB[d] + np.arange(C)
    out = np.zeros((D1, D2, B, SOUT), np.float32)
    for c in range(NCORES):
        for b in range(BLOC):
            hoc = results[c][f"ho{b}"][:, valid]  # (128, 16384) fp16
            out[I, J, BLOC * c + b, :] = hoc.T.astype(np.float32)
    return out


def kernel(x, w, u, bias, _trace=False):
    from concourse.bass_utils import run_bass_kernel_spmd

    x = np.asarray(x, dtype=np.float32)
    w = np.asarray(w, dtype=np.float32)
    u = np.asarray(u, dtype=np.float32)
    bias = np.asarray(bias, dtype=np.float32)

    nc = _build_program()
    in_maps = _prep_inputs(x, w, u, bias)
    res = run_bass_kernel_spmd(
        nc, in_maps, core_ids=list(range(NCORES)), trace=_trace
    )
    _CACHE["last_result"] = res
    return _assemble(res.results)


# revision 25
# speedup vs baseline: 1.1018x; 1.0052x over previous
"""MDRNN 2D-grid recurrence kernel for 8 Trainium2 NeuronCores.

h[i,j] = tanh(x[i,j] @ w + h[i-1,j]*u0 + h[i,j-1]*u1 + bias)

Strategy (v2):
  - Data-parallel over batch: B=16 -> 2 batch elements per core, run as two
    INDEPENDENT anti-diagonal wavefront chains interleaved on the engines
    (decouples the serial dependency chains; engines stay saturated).
  - fp16 storage for x, w, h; fp32 PSUM/z accumulation.
  - GEMM (w stationary, K=64, fp16) runs ahead of the wavefront into PSUM
    chunks aligned to whole diagonals; the per-channel bias is folded into
    the tanh's per-partition bias operand (no ones-row).
  - Per diagonal d of chain b (C cells):
      PE : psum[:, diag] += diag(u0) @ stage_b[up-slice]   (fp16 matmul)
      PE : psum[:, diag] += diag(u1) @ stage_b[left-slice] (fp16 matmul)
      ACT: stage_b[:, d] = tanh(psum[:, diag] + bias)      (fp16 out)
    The two matmuls pipeline back-to-back on the PE; DVE is unused, so the
    serial chain per diagonal is tanh -> mm,mm -> tanh.
  - stage is gap-padded (1 zero col between diagonals) so up/left reads are
    plain shifted slices with boundary zeros from the gaps.
  - Output DMA per 2048-col segment; host inverse-permutes and casts fp32.
"""

import numpy as np

D1, D2, B, SIN, SOUT = 128, 128, 16, 64, 128
NCORES = 8
BLOC = B // NCORES  # 2 chains per core
ND = D1 + D2 - 1  # 255
NC1 = D1 * D2  # 16384 packed cols per chain
SEG = 2048
CHUNK = 512  # psum bank cols


def _geom():
    geo, pb, gb = [], [0], [1]
    for d in range(ND):
        i0 = max(0, d - (D2 - 1))
        i1 = min(D1 - 1, d)
        C = i1 - i0 + 1
        geo.append((i0, C))
        pb.append(pb[-1] + C)
        gb.append(gb[-1] + C + 1)
    return geo, pb, gb


_GEO, _PB, _GB = _geom()
NCG1 = _GB[-1]  # 16640
NSEG = (NCG1 + SEG - 1) // SEG


def _chunks(first_len):
    # <=2 diagonals per chunk: the gemm matmul (<=256 cols, ~340ns) then
    # fits inside a single tanh-wait window on the PE and never delays the
    # wavefront's chain matmuls. Each chunk still gets a full 2KB bank tile
    # (PSUM start=True marks the whole bank pending-zero, so chunks must
    # never share a bank). `first_len` staggers the two chains' chunk
    # boundaries so each PE idle window absorbs only one chain's gemm.
    out = []
    d0 = 0
    nxt = first_len
    while d0 < ND:
        pc0 = _PB[d0]
        d1 = d0
        while d1 + 1 < ND and (d1 - d0 + 1) < nxt and _PB[d1 + 2] - pc0 <= CHUNK:
            d1 += 1
        out.append((d0, d1, pc0, _PB[d1 + 1] - pc0))
        d0 = d1 + 1
        nxt = 2
    return out


_CHUNKS_B = [_chunks(2), _chunks(1)]
_CHUNK_OF_B = []
for _ch in _CHUNKS_B:
    _m = {}
    for _ci, (_a, _b, _, _) in enumerate(_ch):
        for _d in range(_a, _b + 1):
            _m[_d] = _ci
    _CHUNK_OF_B.append(_m)


def _diag_order():
    I, J = [], []
    for d in range(ND):
        for i in range(max(0, d - (D2 - 1)), min(D1 - 1, d) + 1):
            I.append(i)
            J.append(d - i)
    return np.array(I), np.array(J)


_CACHE = {}


def _build_program():
    if "nc" in _CACHE:
        return _CACHE["nc"]
    import concourse.mybir as mybir
    from concourse import bacc
    import concourse.bass as bass
    from concourse.tile import TileContext

    f32 = mybir.dt.float32
    f16 = mybir.dt.float16
    Tanh = mybir.ActivationFunctionType.Tanh

    nc = bacc.Bacc(None, target_bir_lowering=False)
    xa = [
        nc.dram_tensor(f"xa{b}", (SIN, NC1), f16, kind="ExternalInput")
        for b in range(BLOC)
    ]
    wg = nc.dram_tensor("wg", (SIN, SOUT), f16, kind="ExternalInput")
    wd = nc.dram_tensor("wd", (SOUT, SOUT), f16, kind="ExternalInput")
    wd1 = nc.dram_tensor("wd1", (SOUT, SOUT), f16, kind="ExternalInput")
    uvb = nc.dram_tensor("uvb", (SOUT, 2), f32, kind="ExternalInput")
    ho = [
        nc.dram_tensor(f"ho{b}", (SOUT, NCG1), f16, kind="ExternalOutput")
        for b in range(BLOC)
    ]

    XSEG = 2048  # x input DMA segment

    with TileContext(nc) as tc:
        with (
            tc.tile_pool(name="const", bufs=1) as constp,
            tc.tile_pool(name="work", bufs=1) as workp,
            tc.tile_pool(name="psum", bufs=8, space=bass.MemorySpace.PSUM) as psump,
        ):
            # DMA order matters for ramp: wg + the first small x pieces go
            # first so gemm chunk 0 can start ASAP; wd/wd1/uvb follow (they
            # gate only the first recurrence matmul / tanh).
            wg_sb = constp.tile([SIN, SOUT], f16, tag="wg")
            nc.sync.dma_start(wg_sb[:], wg[:])
            wd_sb = constp.tile([SOUT, SOUT], f16, tag="wd")
            wd1_sb = constp.tile([SOUT, SOUT], f16, tag="wd1")
            u_sb = constp.tile([SOUT, 2], f32, tag="uvb")
            bias = u_sb[:, 1:2]

            x_sb, stage = [], []
            for b in range(BLOC):
                xt = constp.tile([SIN, NC1], f16, tag=f"x{b}", name=f"x_sb{b}")
                x_sb.append(xt)
                st = workp.tile([SOUT, NCG1], f16, tag=f"st{b}", name=f"stage{b}")
                stage.append(st)
            nc.sync.dma_start(wd_sb[:], wd[:])
            nc.sync.dma_start(wd1_sb[:], wd1[:])
            nc.sync.dma_start(u_sb[:], uvb[:])
            for b in range(BLOC):
                nc.sync.dma_start(x_sb[b][:, 0:512], xa[b][:, 0:512])
            xsegs = [(512, XSEG)] + [
                (s, s + XSEG) for s in range(XSEG, NC1, XSEG)
            ]
            for lo, hi in xsegs:
                for b in range(BLOC):
                    nc.sync.dma_start(x_sb[b][:, lo:hi], xa[b][:, lo:hi])
            for s in range(NSEG):
                lo = s * SEG
                hi = min(lo + SEG, NCG1)
                for b in range(BLOC):
                    nc.gpsimd.memset(stage[b][:, lo:hi], 0.0)

            # gemm chunk emission (ahead of the wavefront)
            pstile = [[None] * len(_CHUNKS_B[b]) for b in range(BLOC)]

            def emit_chunk(b, ci):
                if pstile[b][ci] is not None:
                    return
                _, _, pc0, ncols = _CHUNKS_B[b][ci]
                ps = psump.tile([SOUT, CHUNK], f32, tag="ps", name="ps")
                nc.tensor.matmul(
                    out=ps[:, :ncols],
                    lhsT=wg_sb[:],
                    rhs=x_sb[b][:, pc0 : pc0 + ncols],
                    start=True,
                    stop=False,
                )
                pstile[b][ci] = ps

            for b in range(BLOC):
                emit_chunk(b, 0)
                emit_chunk(b, 1)

            seg_done = [0] * BLOC
            for d in range(ND):
                i0, C = _GEO[d]
                gbd = _GB[d]
                if d == 0:
                    hls = hus = 0
                elif _GEO[d - 1][0] == i0:
                    hls = _GB[d - 1]
                    hus = _GB[d - 1] - 1
                else:
                    hls = _GB[d - 1] + 1
                    hus = _GB[d - 1]
                for b in range(BLOC):
                    ci = _CHUNK_OF_B[b][d]
                    poff = _PB[d] - _CHUNKS_B[b][ci][2]
                    crossing = ci + 1 < len(_CHUNKS_B[b]) and _CHUNK_OF_B[b].get(
                        d + 1, -1
                    ) != ci
                    ps = pstile[b][ci]
                    nc.tensor.matmul(
                        out=ps[:, poff : poff + C],
                        lhsT=wd_sb[:],
                        rhs=stage[b][:, hus : hus + C],
                        start=False,
                        stop=False,
                        skip_group_check=True,
                    )
                    nc.tensor.matmul(
                        out=ps[:, poff : poff + C],
                        lhsT=wd1_sb[:],
                        rhs=stage[b][:, hls : hls + C],
                        start=False,
                        stop=True,
                        skip_group_check=True,
                    )
                    # prefetch this chain's next gemm chunk in the PE idle
                    # window right behind this chain's recurrence matmuls
                    if crossing:
                        emit_chunk(b, ci + 1)
                    nc.scalar.activation(
                        out=stage[b][:, gbd : gbd + C],
                        in_=ps[:, poff : poff + C],
                        func=Tanh,
                        bias=bias,
                    )
                for b in range(BLOC):
                    while (seg_done[b] + 1) * SEG <= gbd:
                        lo = seg_done[b] * SEG
                        nc.sync.dma_start(
                            ho[b][:, lo : lo + SEG], stage[b][:, lo : lo + SEG]
                        )
                        seg_done[b] += 1
            for b in range(BLOC):
                while seg_done[b] * SEG < NCG1:
                    lo = seg_done[b] * SEG
                    hi = min(lo + SEG, NCG1)
                    nc.sync.dma_start(ho[b][:, lo:hi], stage[b][:, lo:hi])
                    seg_done[b] += 1

    nc.compile()
    _CACHE["nc"] = nc
    return nc


def _prep_inputs(x, w, u, bias):
    I, J = _diag_order()
    xd = np.ascontiguousarray(x[I, J])  # (16384, B, SIN) fp32
    wg = w.astype(np.float16)
    wd = np.diag(u[0]).astype(np.float16)
    wd1 = np.diag(u[1]).astype(np.float16)
    uvb = np.stack([u[1], bias], axis=1).astype(np.float32)  # (128, 2)
    in_maps = []
    for c in range(NCORES):
        m = {"wg": wg, "wd": wd, "wd1": wd1, "uvb": uvb}
        for b in range(BLOC):
            xc = xd[:, BLOC * c + b, :]  # (16384, 64)
            m[f"xa{b}"] = np.ascontiguousarray(xc.T.astype(np.float16))
        in_maps.append(m)
    return in_maps


def _assemble(results):
    I, J = _diag_order()
    valid = np.zeros(NC1, np.int64)
    for d in range(ND):
        C = _GEO[d][1]
        valid[_PB[d] : _PB[d] + C] = _GB[d] + np.arange(C)
    out = np.zeros((D1, D2, B, SOUT), np.float32)
    for c in range(NCORES):
        for b in range(BLOC):
            hoc = results[c][f"ho{b}"][:, valid]  # (128, 16384) fp16
            out[I, J, BLOC * c + b, :] = hoc.T.astype(np.float32)
    return out


def kernel(x, w, u, bias, _trace=False):
    from concourse.bass_utils import run_bass_kernel_spmd

    x = np.asarray(x, dtype=np.float32)
    w = np.asarray(w, dtype=np.float32)
    u = np.asarray(u, dtype=np.float32)
    bias = np.asarray(bias, dtype=np.float32)

    nc = _build_program()
    in_maps = _prep_inputs(x, w, u, bias)
    res = run_bass_kernel_spmd(
        nc, in_maps, core_ids=list(range(NCORES)), trace=_trace
    )
    _CACHE["last_result"] = res
    return _assemble(res.results)
